# revision 1
# baseline (speedup 1.0000x reference)
"""Trainium2 Bass kernel for MockMobGatedDeltaNetMoE.

Sharding: head-parallel over H=8 heads, one head per NeuronCore.
Each core computes its head's full contribution (projections, routing,
ratio-expert attention, gated combine, output projection partial-sum);
the host sums the 8 partial outputs.

Math notes (exact-equivalent reformulations of the reference):
 - softmax(x) ratios computed from exp(x) directly (no max-subtract; logits
   are ~N(0,1) so exp is safe in fp32).
 - router: top-2 of 4 via two reduce_max passes; weights s_i/(2*(m1+m2)).
 - attention: masked keys contribute exp(0)=1 to the denominator and 0 to
   the numerator. We compute exp(S/16 - 30*(1-m_k)) (masked keys -> ~1e-13),
   and add back cnt = #masked keys to the denominator via a ones-matmul.
 - per-(r,q) combine scalar c = rw / denom folded into PSUM eviction.
All matmuls run as float32r (full fp32 data, fast PE mode).
"""

import numpy as np

import concourse.bass as bass
import concourse.bacc as bacc
import concourse.tile as tile
from concourse import mybir
from concourse.bass_utils import run_bass_kernel_spmd

F32 = mybir.dt.float32
F32R = mybir.dt.float32r
BF16 = mybir.dt.bfloat16
ALU = mybir.AluOpType
ACTF = mybir.ActivationFunctionType
AX = mybir.AxisListType

H, D, R, NE = 8, 256, 6, 4          # heads, head_dim, experts, routed experts
HID, DV, T = 2048, 512, 2048        # hidden, head_v_dim, b*t tokens
NB = 2                              # batch
TB = T // NB                        # tokens per batch (attention window)
SCALE = 1.0 / 16.0                  # 1/sqrt(D)
NEG = -30.0                         # masked-key logit bias


def _r(ap):
    return ap


def _body(ctx, nc, tc, io):
    hsT, wq, wk, wv, wg, wqe, wke, hsh, hsl, wfh, wfl, wo, out = io
    import os
    SKIP = set(os.environ.get("KSKIP", "").split(","))

    import contextlib

    const = ctx.enter_context(tc.tile_pool(name="const", bufs=1))
    pers = ctx.enter_context(tc.tile_pool(name="pers", bufs=1))

    ones_f32 = const.tile([128, 128], F32, name="ones_f32")
    nc.vector.memset(ones_f32[:], 1.0)
    ones2 = const.tile([128, 2], F32R, name="ones2")
    nc.scalar.copy(ones2[:], ones_f32[:, 0:2])
    ones128 = const.tile([128, 128], F32R, name="ones128")
    nc.scalar.copy(ones128[:], ones_f32[:])
    from concourse.masks import make_identity
    ident = const.tile([128, 128], F32, name="ident")
    make_identity(nc, ident)
    # fused routing weight (Wq_head @ Wgate, host-fp64) split hi/lo bf16
    wfh_sb = const.tile([128, 64], BF16, name="wfh_sb")
    wfl_sb = const.tile([128, 64], BF16, name="wfl_sb")
    for hc in range(16):
        nc.sync.dma_start(out=wfh_sb[:, hc * 4:(hc + 1) * 4],
                          in_=wfh[hc * 128:(hc + 1) * 128, :])
        nc.sync.dma_start(out=wfl_sb[:, hc * 4:(hc + 1) * 4],
                          in_=wfl[hc * 128:(hc + 1) * 128, :])
    logit_sb = pers.tile([128, 64], F32, name="logit_sb")

    # persistent tensors (col-blocked single tiles)
    qT = pers.tile([128, 2 * T], F32R, name="qT")        # [d-chunk, token]
    kT = pers.tile([128, 2 * T], F32R, name="kT")
    wqe_sb = pers.tile([128, 2 * 1536], F32R, name="wqe_sb")
    wke_sb = pers.tile([128, 2 * 1536], F32R, name="wke_sb")
    rw_all = pers.tile([128, 16 * R], F32, name="rw_all")
    biasN = pers.tile([128, 16 * NE], F32, name="biasN")
    invm = pers.tile([128, 16 * NE], F32R, name="invm")
    for dc in range(2):
        nc.sync.dma_start(out=wqe_sb[:, dc * 1536:(dc + 1) * 1536],
                          in_=wqe[dc * 128:(dc + 1) * 128, :])
        nc.sync.dma_start(out=wke_sb[:, dc * 1536:(dc + 1) * 1536],
                          in_=wke[dc * 128:(dc + 1) * 128, :])

    dram = ctx.enter_context(tc.tile_pool(name="dram", bufs=1, space="DRAM"))
    v_dram = dram.tile([T, DV], F32R, name="v_dram")
    g_dram = dram.tile([T, DV], F32, name="g_dram")

    # ---------------- phase 1: projections ----------------
    with tc.tile_pool(name="p1", bufs=1) as p1, \
         tc.tile_pool(name="p1ps", bufs=1, space="PSUM") as p1ps:
        for tb in range(4):  # token blocks of 512
            t0 = tb * 512
            hst = []
            for hc in range(16):
                ht = p1.tile([128, 512], F32R, name="hst", tag="hst", bufs=17)
                nc.sync.dma_start(out=ht[:], in_=hsT[hc * 128:(hc + 1) * 128, t0:t0 + 512])
                hst.append(ht)
            for wsrc, dstT in (() if "p1q" in SKIP else ((wq, qT), (wk, kT))):
                wt = []
                for hc in range(16):
                    w1 = p1.tile([128, 256], F32R, name="wt", tag="wt", bufs=17)
                    nc.sync.dma_start(out=w1[:], in_=wsrc[hc * 128:(hc + 1) * 128, :])
                    wt.append(w1)
                for f in range(2):
                    ps = p1ps.tile([128, 512], F32, name="psq", tag="psq", bufs=2)
                    for hc in range(16):
                        nc.tensor.matmul(ps[:], _r(wt[hc][:, f * 128:(f + 1) * 128]),
                                         _r(hst[hc][:]), start=(hc == 0), stop=(hc == 15))
                    nc.scalar.copy(dstT[:, f * T + t0:f * T + t0 + 512], ps[:])
            # routing logits: 3-term bf16 split-GEMM (exact products, fp32 accum)
            hih, hil = [], []
            for hc in (() if "p1r" in SKIP else range(16)):
                h1 = p1.tile([128, 512], BF16, name="hih", tag="hih", bufs=17)
                nc.sync.dma_start(out=h1[:], in_=hsh[hc * 128:(hc + 1) * 128, t0:t0 + 512])
                hih.append(h1)
                h2 = p1.tile([128, 512], BF16, name="hil", tag="hil", bufs=17)
                nc.sync.dma_start(out=h2[:], in_=hsl[hc * 128:(hc + 1) * 128, t0:t0 + 512])
                hil.append(h2)
            for tl in (() if "p1r" in SKIP else range(4)):
                tt = tb * 4 + tl
                psr = p1ps.tile([128, 4], F32, name="psr", tag="psr", bufs=2)
                n_mm = 0
                for aa, bb in ((hih, wfh_sb), (hih, wfl_sb), (hil, wfh_sb)):
                    for hc in range(16):
                        nc.tensor.matmul(psr[:],
                                         aa[hc][:, tl * 128:(tl + 1) * 128],
                                         bb[:, hc * 4:(hc + 1) * 4],
                                         start=(n_mm == 0), stop=(n_mm == 47))
                        n_mm += 1
                nc.scalar.copy(logit_sb[:, tt * 4:(tt + 1) * 4], psr[:])
            for wsrc, ddst in (() if "p1vg" in SKIP else ((wv, v_dram), (wg, g_dram))):
                wt = []
                for hc in range(16):
                    w1 = p1.tile([128, 512], F32R, name="wt", tag="wt", bufs=17)
                    nc.sync.dma_start(out=w1[:], in_=wsrc[hc * 128:(hc + 1) * 128, :])
                    wt.append(w1)
                for tt4 in range(4):
                    ps = p1ps.tile([128, 512], F32, name="psv", tag="psv", bufs=2)
                    for hc in range(16):
                        nc.tensor.matmul(ps[:], _r(hst[hc][:, tt4 * 128:(tt4 + 1) * 128]),
                                         _r(wt[hc][:]), start=(hc == 0), stop=(hc == 15))
                    st = p1.tile([128, 512], F32R if ddst is v_dram else F32, name="vgst", tag="vgst", bufs=4)
                    nc.scalar.copy(st[:], ps[:])
                    nc.sync.dma_start(out=ddst[t0 + tt4 * 128:t0 + tt4 * 128 + 128, :], in_=st[:])

    # ---------------- phase 2: routing ----------------
    nc.vector.memset(rw_all[:], 0.25)
    with tc.tile_pool(name="p2", bufs=4) as p2:
        for tt in (() if "p2" in SKIP else range(16)):
            lg = logit_sb[:, tt * 4:(tt + 1) * 4]
            s = p2.tile([128, 4], F32, name="s")
            nc.scalar.activation(s[:], lg, ACTF.Exp)
            m1 = p2.tile([128, 1], F32, name="m1")
            nc.vector.tensor_reduce(m1[:], lg, axis=AX.X, op=ALU.max)
            eq = p2.tile([128, 4], F32, name="eq")
            nc.vector.tensor_scalar(eq[:], lg, m1[:], None, ALU.is_ge)
            sm = p2.tile([128, 4], F32, name="sm")
            nc.vector.scalar_tensor_tensor(sm[:], eq[:], -1e30, lg, ALU.mult, ALU.add)
            m2 = p2.tile([128, 1], F32, name="m2")
            nc.vector.tensor_reduce(m2[:], sm[:], axis=AX.X, op=ALU.max)
            sel = p2.tile([128, 4], F32, name="sel")
            nc.vector.tensor_scalar(sel[:], lg, m2[:], None, ALU.is_ge)
            w4 = p2.tile([128, 4], F32, name="w4")
            nc.vector.tensor_tensor(w4[:], s[:], sel[:], ALU.mult)
            den = p2.tile([128, 1], F32, name="den")
            nc.vector.tensor_reduce(den[:], w4[:], axis=AX.X, op=ALU.add)
            dinv = p2.tile([128, 1], F32, name="dinv")
            nc.vector.reciprocal(dinv[:], den[:])
            nc.vector.tensor_scalar(rw_all[:, tt * R + 2:tt * R + 6], w4[:], dinv[:], 0.5,
                                    ALU.mult, ALU.mult)
            nc.vector.tensor_scalar(biasN[:, tt * NE:(tt + 1) * NE], sel[:], 30.0, -30.0,
                                    ALU.mult, ALU.add)
            nc.vector.tensor_scalar(invm[:, tt * NE:(tt + 1) * NE], sel[:], -1.0, 1.0,
                                    ALU.mult, ALU.add)

    # ---------------- phase 3: expert attention ----------------
    pers3 = ctx.enter_context(tc.tile_pool(name="pers3", bufs=1))
    o_acc = pers3.tile([128, 16 * DV], F32, name="o_acc")  # per t-tile block
    with tc.tile_pool(name="p3", bufs=1) as p3, \
         tc.tile_pool(name="p3ps", bufs=1, space="PSUM") as p3ps:
        # masked-key counts per (b, routed expert): cnt[b][:, e] = #inactive keys
        cnt_sb = pers3.tile([128, 2 * NE], F32, name="cnt_sb")
        for b in range(NB):
            pscnt = p3ps.tile([128, 4], F32, name="pscnt", tag="pscnt", bufs=1)
            for kt in range(8):
                ktt = b * 8 + kt
                nc.tensor.matmul(pscnt[:], ones128[:],
                                 invm[:, ktt * NE:(ktt + 1) * NE],
                                 start=(kt == 0), stop=(kt == 7))
            nc.scalar.copy(cnt_sb[:, b * NE:(b + 1) * NE], pscnt[:])
        for r in range(R):
            # expansions qeT[r], keT[r]: [256 e, 2048 t] as 2 chunk tiles
            qeT, keT = [], []
            for wsb, lst, nm in (() if "exp" in SKIP else ((wqe_sb, qeT, "qeTt"), (wke_sb, keT, "keTt"))):
                for dco in range(2):
                    et = p3.tile([128, T], F32R, name=nm, tag=nm, bufs=3)
                    lst.append(et)
                    for nb4 in range(4):
                        ps = p3ps.tile([128, 512], F32, name="psqe", tag="psqe", bufs=2)
                        for dci in range(2):
                            nc.tensor.matmul(
                                ps[:],
                                _r(wsb[:, dci * 1536 + r * 256 + dco * 128:
                                       dci * 1536 + r * 256 + dco * 128 + 128]),
                                _r(qT[:, dci * T + nb4 * 512:dci * T + nb4 * 512 + 512]
                                   if nm == "qeTt" else
                                   kT[:, dci * T + nb4 * 512:dci * T + nb4 * 512 + 512]),
                                start=(dci == 0), stop=(dci == 1))
                        nc.scalar.copy(et[:, nb4 * 512:nb4 * 512 + 512], ps[:])
            for b in (() if "att" in SKIP else range(NB)):
                boff = b * TB
                vks = []
                for kt in range(8):
                    vt = p3.tile([128, DV], F32R, name="vks", tag="vks", bufs=10)
                    nc.sync.dma_start(out=vt[:], in_=v_dram[boff + kt * 128:boff + kt * 128 + 128, :])
                    vks.append(vt)
                for half in range(2):
                    qoff = boff + half * 512
                    expS = []
                    for kt in range(8):
                        ktt = b * 8 + kt
                        pss = p3ps.tile([128, 512], F32, name="pss", tag="pss", bufs=2)
                        for dc in range(2):
                            nc.tensor.matmul(
                                pss[:],
                                _r(keT[dc][:, boff + kt * 128:boff + kt * 128 + 128]),
                                _r(qeT[dc][:, qoff:qoff + 512]),
                                start=(dc == 0), stop=(dc == 1))
                        es = p3.tile([128, 512], F32R, name="expS", tag="expS", bufs=10)
                        if r >= 2:
                            nc.scalar.activation(
                                es[:], pss[:], ACTF.Exp, scale=SCALE,
                                bias=biasN[:, ktt * NE + (r - 2):ktt * NE + (r - 2) + 1])
                        else:
                            nc.scalar.activation(es[:], pss[:], ACTF.Exp, scale=SCALE)
                        expS.append(es)
                    psden = p3ps.tile([128, 8], F32, name="psden", tag="psden", bufs=1)
                    for j in range(4):
                        for kt in range(8):
                            nc.tensor.matmul(psden[:, 2 * j:2 * j + 2],
                                             expS[kt][:, j * 128:j * 128 + 128],
                                             ones2[:],
                                             start=(kt == 0), stop=(kt == 7))
                    for j in range(4):
                        pso = p3ps.tile([128, 512], F32, name="pso", tag="pso", bufs=2)
                        for kt in range(8):
                            nc.tensor.matmul(pso[:],
                                             _r(expS[kt][:, j * 128:j * 128 + 128]),
                                             _r(vks[kt][:]),
                                             start=(kt == 0), stop=(kt == 7))
                        tt = b * 8 + half * 4 + j
                        dinv = p3.tile([128, 1], F32, name="adinv", tag="adinv", bufs=4)
                        if r >= 2:
                            dtot = p3.tile([128, 1], F32, name="dtot", tag="dtot", bufs=4)
                            nc.vector.tensor_tensor(
                                dtot[:], psden[:, 2 * j:2 * j + 1],
                                cnt_sb[:, b * NE + (r - 2):b * NE + (r - 2) + 1], ALU.add)
                            nc.vector.reciprocal(dinv[:], dtot[:])
                        else:
                            nc.vector.reciprocal(dinv[:], psden[:, 2 * j:2 * j + 1])
                        cmul = p3.tile([128, 1], F32, name="cmul", tag="cmul", bufs=4)
                        nc.vector.tensor_tensor(cmul[:], dinv[:],
                                                rw_all[:, tt * R + r:tt * R + r + 1], ALU.mult)
                        if r == 0:
                            nc.vector.tensor_scalar(o_acc[:, tt * DV:(tt + 1) * DV],
                                                    pso[:], cmul[:], None, ALU.mult)
                        else:
                            tmp = p3.tile([128, DV], F32, name="otmp", tag="otmp", bufs=3)
                            nc.vector.tensor_scalar(tmp[:], pso[:], cmul[:], None, ALU.mult)
                            nc.vector.tensor_tensor(o_acc[:, tt * DV:(tt + 1) * DV],
                                                    o_acc[:, tt * DV:(tt + 1) * DV],
                                                    tmp[:], ALU.add)

    # ---------------- phase 4: gate, transpose, output projection ----------------
    with tc.tile_pool(name="p4", bufs=1) as p4, \
         tc.tile_pool(name="p4ps", bufs=1, space="PSUM") as p4ps:
        if "p4" in SKIP:
            return
        wo_sb = [p4.tile([128, HID], F32R, name=f"wo_sb{i}", tag=f"wo_sb{i}") for i in range(4)]
        for i in range(4):
            nc.sync.dma_start(out=wo_sb[i][:], in_=wo[i * 128:(i + 1) * 128, :])
        Xt = [p4.tile([128, T], F32R, name=f"xt{i}", tag=f"xt{i}") for i in range(4)]
        for tt in range(16):
            gsb = p4.tile([128, DV], F32, name="gsb", tag="gsb", bufs=3)
            nc.sync.dma_start(out=gsb[:], in_=g_dram[tt * 128:(tt + 1) * 128, :])
            sg = p4.tile([128, DV], F32, name="sg", tag="sg", bufs=3)
            nc.scalar.activation(sg[:], gsb[:], ACTF.Sigmoid)
            nc.vector.tensor_tensor(sg[:], sg[:], gsb[:], ALU.mult)  # silu(g)
            xres = p4.tile([128, DV], F32, name="xres", tag="xres", bufs=3)
            nc.vector.tensor_tensor(xres[:], o_acc[:, tt * DV:(tt + 1) * DV], sg[:], ALU.mult)
            for dvc in range(4):
                pst = p4ps.tile([128, 128], F32, name="pst", tag="pst", bufs=2)
                nc.tensor.transpose(pst[:], xres[:, dvc * 128:(dvc + 1) * 128], ident[:])
                nc.scalar.copy(Xt[dvc][:, tt * 128:(tt + 1) * 128], pst[:])
        for tt in range(16):
            for hb in range(4):
                psf = p4ps.tile([128, 512], F32, name="psf", tag="psf", bufs=2)
                for dvc in range(4):
                    nc.tensor.matmul(psf[:], _r(Xt[dvc][:, tt * 128:(tt + 1) * 128]),
                                     _r(wo_sb[dvc][:, hb * 512:(hb + 1) * 512]),
                                     start=(dvc == 0), stop=(dvc == 3))
                ost = p4.tile([128, 512], F32, name="ost", tag="ost", bufs=4)
                nc.scalar.copy(ost[:], psf[:])
                nc.sync.dma_start(out=out[tt * 128:(tt + 1) * 128, hb * 512:(hb + 1) * 512],
                                  in_=ost[:])


_PROGRAM = None


def build_program():
    global _PROGRAM
    if _PROGRAM is not None:
        return _PROGRAM
    from contextlib import ExitStack
    nc = bacc.Bacc("TRN2", target_bir_lowering=False, debug=False, num_devices=8)
    names = [("hsT", [HID, T], F32R), ("wq", [HID, D], F32R), ("wk", [HID, D], F32R),
             ("wv", [HID, DV], F32R), ("wg", [HID, DV], F32R), ("wqe", [D, D * R], F32R),
             ("wke", [D, D * R], F32R), ("hsh", [HID, T], BF16), ("hsl", [HID, T], BF16),
             ("wfh", [HID, NE], BF16), ("wfl", [HID, NE], BF16), ("wo", [DV, HID], F32R)]
    io = [nc.dram_tensor(n, s, dt, kind="ExternalInput").ap() for n, s, dt in names]
    io.append(nc.dram_tensor("out", [T, HID], F32, kind="ExternalOutput").ap())
    with tile.TileContext(nc) as tc:
        from contextlib import ExitStack as ES
        with ES() as ctx:
            _body(ctx, nc, tc, io)
    nc.compile()
    _PROGRAM = nc
    return nc


def make_in_maps(hidden_states, Wq, Wk, Wv, Wq_exp, Wk_exp, Wgate, Wg, Wo):
    import ml_dtypes
    bf = ml_dtypes.bfloat16
    hs2 = np.asarray(hidden_states, np.float32).reshape(T, HID)
    hsT = np.ascontiguousarray(hs2.T)
    hsh = np.ascontiguousarray(hsT.astype(bf))
    hsl = np.ascontiguousarray((hsT.astype(np.float64) - hsh.astype(np.float64)).astype(bf))
    wfus = []
    for c in range(8):
        wfu = (np.asarray(Wq, np.float64)[:, c * D:(c + 1) * D]
               @ np.asarray(Wgate, np.float64))
        wfh = wfu.astype(bf)
        wfl = (wfu - wfh.astype(np.float64)).astype(bf)
        wfus.append((np.ascontiguousarray(wfh), np.ascontiguousarray(wfl)))
    in_maps = []
    for c in range(8):
        in_maps.append({
            "hsT": hsT,
            "wq": np.ascontiguousarray(np.asarray(Wq, np.float32)[:, c * D:(c + 1) * D]),
            "wk": np.ascontiguousarray(np.asarray(Wk, np.float32)[:, c * D:(c + 1) * D]),
            "wv": np.ascontiguousarray(np.asarray(Wv, np.float32)[:, c * DV:(c + 1) * DV]),
            "wg": np.ascontiguousarray(np.asarray(Wg, np.float32)[:, c * DV:(c + 1) * DV]),
            "wqe": np.ascontiguousarray(np.asarray(Wq_exp, np.float32)[c]),
            "wke": np.ascontiguousarray(np.asarray(Wk_exp, np.float32)[c]),
            "hsh": hsh, "hsl": hsl,
            "wfh": wfus[c][0], "wfl": wfus[c][1],
            "wo": np.ascontiguousarray(np.asarray(Wo, np.float32)[c * DV:(c + 1) * DV, :]),
        })
    return in_maps


def kernel(hidden_states, Wq, Wk, Wv, Wq_exp, Wk_exp, Wgate, Wg, Wo):
    nc = build_program()
    in_maps = make_in_maps(hidden_states, Wq, Wk, Wv, Wq_exp, Wk_exp, Wgate, Wg, Wo)
    res = run_bass_kernel_spmd(nc, in_maps, list(range(8))).results
    out = np.zeros((T, HID), np.float32)
    for c in range(8):
        out += res[c]["out"]
    return out.reshape(2, 1024, HID).astype(np.float32)

